# revision 1
# baseline (speedup 1.0000x reference)
"""Trainium2 Bass kernel for nn_AssociatorLoss.

Reference computation (B=32, N=32), a = cayley_cube (B,N,N,N), rows a[b,i,j,:]
are probability distributions:

    one[b,i,j,k,l] = sum_m a[b,i,m,l] * a[b,j,k,m]
    two[b,i,j,k,l] = sum_m a[b,m,k,l] * a[b,i,j,m]
    kl = sum(two * (log(two) - log(one))) / B

Strategy (data-parallel over b, 4 batch elements per core, no collectives —
the 8 per-core partial sums are combined on the host):

Per batch element, with x,y,z the three trailing axes of a[b]:
  A  = a[b] in SBUF as [x, (y,z)]        (natural, 32 partitions x 1024)
  AT = 32x32-block transpose of A  -> [z, (y,x)]
  AY = block transpose of A viewed with free dims swapped -> [y, (z,x)]

Matmuls (K = m = 32, bf16, PE):
  two  chunk c (i in [4c,4c+4)):  out[p=(i,j), f=(k,l)] :
       lhsT[m,(i,j)] = AT viewed [z,x,y][:, 4c:4c+4, :],  rhs[m,(k,l)] = A
  one  chunk c:                   out[p=(i,l), f=(k,j)] :
       lhsT[m,(i,l)] = AY viewed [y,x,z][:, 4c:4c+4, :],  rhs[m,(k,j)] = AT

  ("one" comes out with free index (k,j) so that the 32x32-block transpose of
   the "two" chunk — which maps [p=(i,j),f=(k,l)] -> [p=(i,l),f=(k,j)] —
   aligns elementwise with it.)

Elementwise/reduction per chunk:
  ACT:    LT = Ln(two_psum) -> bf16, LO = Ln(one_psum) -> bf16
  DVE:    twot = block-transpose(two_psum);  ttr: P = two_psum*LT, rowsum -> accP
  GPSIMD: stt:  P2 = twot*LO, rowsum -> accM
KL = (sum(accP) - sum(accM)) / B, finished on host in float64.
"""

import sys

for _p in ("/opt/trn_rl_repo",):
    if _p not in sys.path:
        sys.path.insert(0, _p)

import numpy as np

import concourse.bacc as bacc
import concourse.mybir as mybir
import concourse.tile as tile
from concourse.bass_utils import run_bass_kernel_spmd

B, N = 32, 32
N_CORES = 8
B_LOCAL = B // N_CORES  # 4
NCHUNK = (N * N) // 128  # 8 chunks of 128 rows per batch element
F32 = mybir.dt.float32
BF16 = mybir.dt.bfloat16
POOL_COLS = 768


def build(b_local=B_LOCAL, mm_dtype=BF16, log_dtype=F32, reps=1, sub_pool=0, skip=(), loop_reps=0):
    nc = bacc.Bacc(None, target_bir_lowering=False)
    ncols = b_local * NCHUNK
    a_ext = nc.declare_dram_parameter("cayley_cube", [b_local, N, N, N], F32, isOutput=False)
    out_ext = nc.declare_dram_parameter("out", [128, ncols], F32, isOutput=True)
    av = a_ext.rearrange("b x y z -> b x (y z)")

    mult = mybir.AluOpType.mult
    add = mybir.AluOpType.add
    subtract = mybir.AluOpType.subtract

    with tile.TileContext(nc) as tc:
        with (
            tc.tile_pool(name="apool", bufs=2) as apool,
            tc.tile_pool(name="spool", bufs=12) as spool,
            tc.tile_pool(name="scratch", bufs=1) as scratch,
            tc.tile_pool(name="acc", bufs=1) as accpool,
            tc.tile_pool(name="psumT", bufs=2, space="PSUM") as psumT,
            tc.tile_pool(name="psumO", bufs=2, space="PSUM") as psumO,
        ):
            accP = accpool.tile([128, ncols], F32)
            p1 = scratch.tile([128, 1024], BF16)

            import contextlib
            loop_ctx = tc.For_i(0, loop_reps, 1) if loop_reps else contextlib.nullcontext()
            with loop_ctx:
             for _rep in range(reps):
              for b in range(b_local):
                # casting DMA: loads f32 from HBM, stores bf16 to SBUF
                ab = apool.tile([N, 1024], mm_dtype, tag="ab")
                nc.gpsimd.dma_start(out=ab[:], in_=av[b])
                # at[z, y*32+x] = a[x,y,z]  (O-matmul rhs: n = k*32+j)
                at = apool.tile([N, 1024], mm_dtype, tag="at")
                nc.vector.transpose(at[:], ab[:])
                # at2[z, x*32+y] = a[x,y,z]  (T-matmul stationary operand:
                # contiguous 128-col slices enumerate (i-group, j))
                at2 = apool.tile([N, 1024], mm_dtype, tag="at2")
                nc.gpsimd.tensor_copy(
                    at2[:].rearrange("p (x y) -> p y x", x=N, y=N),
                    at[:].rearrange("p (y x) -> p y x", y=N, x=N),
                )
                # ay2[y, x*32+z] = a[x,y,z]  (O-matmul stationary operand)
                ay2 = apool.tile([N, 1024], mm_dtype, tag="ay2")
                nc.vector.transpose(ay2[:], at2[:])

                for c in range(NCHUNK):
                    col = b * NCHUNK + c
                    tp = psumT.tile([128, 1024], F32, tag="tp")
                    op = psumO.tile([128, 1024], F32, tag="op")
                    ms = slice(128 * c, 128 * (c + 1))
                    for h in range(2):
                        cs = slice(512 * h, 512 * (h + 1))
                        nc.tensor.matmul(op[:, cs], ay2[:, ms], at[:, cs],
                                         start=True, stop=True)
                    for h in range(2):
                        cs = slice(512 * h, 512 * (h + 1))
                        nc.tensor.matmul(tp[:, cs], at2[:, ms], ab[:, cs],
                                         start=True, stop=True)

                    # ACT: the two mandatory Ln passes (bf16 out), ln(one) first
                    # so the DVE transpose unblocks as early as possible
                    lo = spool.tile([128, 1024], BF16, tag="lo")
                    nc.scalar.activation(lo[:], op[:], mybir.ActivationFunctionType.Ln)
                    lt = spool.tile([128, 1024], BF16, tag="lt")
                    nc.scalar.activation(lt[:], tp[:], mybir.ActivationFunctionType.Ln)

                    # align ln(one) with two's layout via 32x32-block transpose
                    lot = spool.tile([128, 1024], BF16, tag="lot")
                    nc.vector.transpose(lot[:], lo[:])
                    # D = ln(two) - ln(one)_aligned on the Pool engine
                    dd = spool.tile([128, 1024], BF16, tag="dd")
                    nc.gpsimd.tensor_tensor(
                        out=dd[:], in0=lt[:], in1=lot[:], op=subtract,
                    )
                    # fused dot: sum two * D -> accP column
                    nc.vector.scalar_tensor_tensor(
                        out=p1[:], in0=tp[:], scalar=1.0, in1=dd[:],
                        op0=mult, op1=mult, accum_out=accP[:, col:col + 1],
                    )

            nc.sync.dma_start(out=out_ext[:, 0:ncols], in_=accP[:])

    nc.compile()
    return nc


def kernel(cayley_cube: np.ndarray) -> np.ndarray:
    assert cayley_cube.shape == (B, N, N, N)
    nc = build()
    shards = cayley_cube.reshape(N_CORES, B_LOCAL, N, N, N)
    in_maps = [
        {"cayley_cube": np.ascontiguousarray(shards[i])} for i in range(N_CORES)
    ]
    res = run_bass_kernel_spmd(nc, in_maps, core_ids=list(range(N_CORES)))
    ncols = B_LOCAL * NCHUNK
    tot = np.float64(0.0)
    for r in res.results:
        acc = r["out"]
        tot += acc[:, :ncols].sum(dtype=np.float64)
    return np.float32(tot / B)


if __name__ == "__main__":
    rng = np.random.default_rng(0)
    raw = rng.uniform(0.05, 1.0, size=(B, N, N, N)).astype(np.float32)
    a = raw / raw.sum(axis=-1, keepdims=True)
    print(kernel(a))



# revision 6
# speedup vs baseline: 1.1006x; 1.1006x over previous
"""Trainium2 Bass kernel for nn_AssociatorLoss (low-rank dot formulation).

Reference (B=32, N=32), a = cayley_cube (B,N,N,N):
    one[b,i,j,k,l] = sum_m a[b,i,m,l] * a[b,j,k,m]
    two[b,i,j,k,l] = sum_m a[b,m,k,l] * a[b,i,j,m]
    kl = sum(two * (log(two) - log(one))) / B

Key identity: in (u=(i,j), v=(k,l)) coordinates two = P·Q with
P[u,m] = a[i,j,m], Q[m,v] = a[m,k,l] (rank 32).  Hence for any X in that
layout:  sum(two ⊙ X) = sum_m,v Q[m,v] · (P^T X)[m,v].
So both dot products reduce to small PE matmuls H = P^T X accumulated over
(i,j)-chunks, with X = ln(two) and X = blockT(ln(one)):

per chunk c (128 (ij)-rows x 1024 (kl)-cols), per batch elem:
  PE : op = one-chunk  [p=(i,l), f=(k,j)]  (2 row-packed K=32 matmuls)
       tp = two-chunk  [p=(i,j), f=(k,l)]  (2 row-packed K=32 matmuls)
  ACT: lt = Ln(tp), lo = Ln(op)   (PSUM -> SBUF bf16)
  DVE: lot = blockT32(lo)         -> [p=(i,j), f=(k,l)]
  PE : H4 quadrants += af_c^T @ {lt,lot}-halves  (4 col-packed matmuls,
       K=128, M=32, accumulated over the 8 chunks in PSUM)
per batch elem:
  DVE: acc[:,b] = rowsum( (abq * sgn) ⊙ H4 )   (sgn=+1 for ln-two quads,
       -1 for ln-one quads); host sums acc and divides by B.

Data-parallel over b: 4 batch elems per core, partial sums combined on host.
"""

import sys

for _p in ("/opt/trn_rl_repo",):
    if _p not in sys.path:
        sys.path.insert(0, _p)

import numpy as np

import concourse.bacc as bacc
import concourse.mybir as mybir
import concourse.tile as tile
from concourse.bass_utils import run_bass_kernel_spmd

B, N = 32, 32
N_CORES = 8
B_LOCAL = B // N_CORES  # 4
NCHUNK = (N * N) // 128  # 8 chunks of 128 (ij)-rows per batch element
F32 = mybir.dt.float32
BF16 = mybir.dt.bfloat16


def build(b_local=B_LOCAL):
    nc = bacc.Bacc(None, target_bir_lowering=False)
    a_ext = nc.declare_dram_parameter("cayley_cube", [b_local, N, N, N], F32, isOutput=False)
    out_ext = nc.declare_dram_parameter("out", [128, b_local], F32, isOutput=True)

    av = a_ext.rearrange("b x y z -> b x (y z)")
    # af rows are (ij)-chunk layout: af[p, c*32+m] = a[4c + p//32, p%32, m]
    av4 = a_ext.rearrange("b (c il) j m -> b (il j) c m", c=NCHUNK, il=4)
    # abq halves: row h*32+m holds a[m, (kl) half h]
    av5 = a_ext.rearrange("b m (h k2) l -> b h m (k2 l)", h=2, k2=16)

    mult = mybir.AluOpType.mult
    Ln = mybir.ActivationFunctionType.Ln

    with tile.TileContext(nc) as tc:
        with (
            tc.tile_pool(name="apool", bufs=2) as apool,
            tc.tile_pool(name="spool", bufs=2) as spool,
            tc.tile_pool(name="scratch", bufs=1) as scratch,
            tc.tile_pool(name="psumO", bufs=2, space="PSUM") as psumO,
            tc.tile_pool(name="psumT", bufs=1, space="PSUM") as psumT,
            tc.tile_pool(name="psumH", bufs=2, space="PSUM") as psumH,
        ):
            sgn = scratch.tile([128, 1], F32)
            nc.vector.memset(sgn[0:64, :], 1.0)
            nc.vector.memset(sgn[64:128, :], -1.0)
            acc = scratch.tile([128, b_local], F32)
            junk = scratch.tile([128, 512], BF16)

            for b in range(b_local):
                # ---- per-batch input prep ----
                ab = apool.tile([N, 1024], BF16, tag="ab")
                nc.gpsimd.dma_start(out=ab[:], in_=av[b])
                af = apool.tile([128, NCHUNK * 32], BF16, tag="af")
                nc.gpsimd.dma_start(
                    out=af[:].rearrange("p (c m) -> p c m", c=NCHUNK, m=N),
                    in_=av4[b],
                )
                abq = apool.tile([128, 512], BF16, tag="abq")
                for q in range(4):
                    nc.gpsimd.dma_start(out=abq[32 * q:32 * q + 32, :],
                                        in_=av5[b, q % 2])

                # at4: [0:32]=at, [32:64]=at, [64:96]=ab, [96:128]=ab
                at4 = apool.tile([128, 1024], BF16, tag="at4")
                nc.vector.transpose(at4[0:32, :], ab[:])      # at[z,(y,x)]
                nc.vector.tensor_copy(at4[32:64, :], at4[0:32, :])
                nc.vector.tensor_copy(at4[64:96, :], ab[:])
                nc.vector.tensor_copy(at4[96:128, :], ab[:])

                # at2[z, x*32+y] from at[z, y*32+x] (strided rearrange copy)
                at2 = apool.tile([N, 1024], BF16, tag="at2")
                nc.gpsimd.tensor_copy(
                    at2[:].rearrange("p (x y) -> p y x", x=N, y=N),
                    at4[0:32, :].rearrange("p (y x) -> p y x", y=N, x=N),
                )
                # ayat4: [0:32]=ay2, [32:64]=ay2, [64:96]=at2, [96:128]=at2
                ayat4 = apool.tile([128, 1024], BF16, tag="ayat4")
                nc.vector.transpose(ayat4[0:32, :], at2[:])   # ay2[y,(x,z)]
                nc.vector.tensor_copy(ayat4[32:64, :], ayat4[0:32, :])
                nc.vector.tensor_copy(ayat4[64:96, :], at2[:])
                nc.vector.tensor_copy(ayat4[96:128, :], at2[:])

                h4 = psumH.tile([128, 512], F32, tag="h4")

                for c in range(NCHUNK):
                    ms = slice(128 * c, 128 * (c + 1))
                    cs = slice(32 * c, 32 * (c + 1))
                    op = psumO.tile([128, 1024], F32, tag="op")
                    tp = psumT.tile([128, 1024], F32, tag="tp")

                    # one: out[p=(i,l), f=(k,j)] = sum_m ay2[m,(i,l)] at[m,(k,j)]
                    nc.tensor.matmul(op[:, 0:512], ayat4[0:32, ms], at4[0:32, 0:512],
                                     start=True, stop=True, tile_position=(0, 0))
                    nc.tensor.matmul(op[:, 512:1024], ayat4[32:64, ms], at4[32:64, 512:1024],
                                     start=True, stop=True, tile_position=(32, 0))
                    # two: out[p=(i,j), f=(k,l)] = sum_m at2[m,(i,j)] ab[m,(k,l)]
                    nc.tensor.matmul(tp[:, 0:512], ayat4[64:96, ms], at4[64:96, 0:512],
                                     start=True, stop=True, tile_position=(64, 0))
                    nc.tensor.matmul(tp[:, 512:1024], ayat4[96:128, ms], at4[96:128, 512:1024],
                                     start=True, stop=True, tile_position=(96, 0))

                    # Ln(tp) first: frees the single-buffered tp pool earlier
                    lt = spool.tile([128, 1024], BF16, tag="lt")
                    nc.scalar.activation(lt[:], tp[:], Ln)
                    lo = spool.tile([128, 1024], BF16, tag="lo")
                    nc.scalar.activation(lo[:], op[:], Ln)

                    lot = spool.tile([128, 1024], BF16, tag="lot")
                    nc.vector.transpose(lot[:], lo[:])

                    st = c == 0
                    sp = c == NCHUNK - 1
                    nc.tensor.matmul(h4[0:32, :], af[:, cs], lt[:, 0:512],
                                     start=st, stop=sp, tile_position=(0, 0))
                    nc.tensor.matmul(h4[32:64, :], af[:, cs], lt[:, 512:1024],
                                     start=st, stop=sp, tile_position=(0, 32))
                    nc.tensor.matmul(h4[64:96, :], af[:, cs], lot[:, 0:512],
                                     start=st, stop=sp, tile_position=(0, 64))
                    nc.tensor.matmul(h4[96:128, :], af[:, cs], lot[:, 512:1024],
                                     start=st, stop=sp, tile_position=(0, 96))

                # drain: acc[:, b] = rowsum((abq*sgn) ⊙ H4)
                nc.vector.scalar_tensor_tensor(
                    out=junk[:], in0=abq[:], scalar=sgn[:, 0:1], in1=h4[:],
                    op0=mult, op1=mult, accum_out=acc[:, b:b + 1],
                )

            nc.sync.dma_start(out=out_ext[:, :], in_=acc[:])

    nc.compile()
    return nc


def kernel(cayley_cube: np.ndarray) -> np.ndarray:
    assert cayley_cube.shape == (B, N, N, N)
    nc = build()
    shards = cayley_cube.reshape(N_CORES, B_LOCAL, N, N, N)
    in_maps = [
        {"cayley_cube": np.ascontiguousarray(shards[i])} for i in range(N_CORES)
    ]
    res = run_bass_kernel_spmd(nc, in_maps, core_ids=list(range(N_CORES)))
    tot = np.float64(0.0)
    for r in res.results:
        tot += r["out"].sum(dtype=np.float64)
    return np.float32(tot / B)


if __name__ == "__main__":
    rng = np.random.default_rng(0)
    raw = rng.uniform(0.05, 1.0, size=(B, N, N, N)).astype(np.float32)
    a = raw / raw.sum(axis=-1, keepdims=True)
    print(kernel(a))
